# revision 1
# baseline (speedup 1.0000x reference)
"""IsoMaxPlus first-part logits kernel for 8 Trainium2 NeuronCores.

reference:
    f = l2norm(features)   [N=16384, D=1024]
    p = l2norm(prototypes) [C=8192, D=1024]
    logits = -|ds| * sqrt(max(2 - 2 * f @ p.T, 1e-12))

Strategy (data-parallel over N, prototypes replicated):
  - Host: shard features over 8 cores (2048 rows each); pre-transpose and
    bf16-cast both operands so everything lands on-device in the layout the
    TensorEngine wants (contraction dim D on partitions). No math happens on
    the host.
  - Device per core:
      * inv_p: column sums of pT^2 via a ones-matmul partition reduction
        (result is broadcast over all 128 partitions for free), then
        x^-1/2 = Exp(-0.5 * Ln(x)) on the Scalar engine.
      * pnT = pT * inv_p  (in-place, DVE, bf16 2x mode)
      * inv_f: row sums of f^2 via one fused tensor_tensor_reduce per tile,
        Sqrt + reciprocal; folded into the post-matmul activation scale.
      * main matmul: out[n,c] accumulated over 8 k-tiles into PSUM
        ([128,512] f32 banks), streaming pnT as the moving operand.
      * post: logits = -sqrt(2ds^2 + (-2ds^2*inv_f[n]) * dot) in one
        ACT Sqrt (per-partition scale/bias) + one DVE negate, then DMA out.
  - max(.., 1e-12) is dropped: 2-2*dot >= 1.5 for this distribution, far
    from the clamp.

Inputs are quantized to bf16 (matching the TensorEngine compute dtype);
measured end-to-end relative error vs the f32 reference is ~1e-4.
"""

import sys

import numpy as np
import ml_dtypes

if "/opt/trn_rl_repo" not in sys.path:
    sys.path.append("/opt/trn_rl_repo")

N, D, C = 16384, 1024, 8192
NCORES = 8
NSH = N // NCORES  # rows per core = 2048
P = 128
NT = NSH // P  # 16 n-tiles per core
KT = D // P  # 8 k-tiles
CG = 2  # c groups
CW = C // CG  # 4096 per group
CB = CW // 512  # 8 chunks of 512 per group

_ctx = {}


def _build_nc():
    import concourse.mybir as mybir
    import concourse.tile as tile
    from concourse import bacc
    from contextlib import ExitStack

    f32 = mybir.dt.float32
    bf16 = mybir.dt.bfloat16
    AF = mybir.ActivationFunctionType

    nc = bacc.Bacc(None, target_bir_lowering=False)

    ftb = nc.dram_tensor("ftb", [NT, P, KT, P], bf16, kind="ExternalInput")
    fnat = nc.dram_tensor("fnat", [NT, P, D], bf16, kind="ExternalInput")
    ptb = nc.dram_tensor("ptb", [KT, P, C], bf16, kind="ExternalInput")
    dsc = nc.dram_tensor("dsc", [1, 1], f32, kind="ExternalInput")
    out = nc.dram_tensor("out", [NSH, C], f32, kind="ExternalOutput")

    with ExitStack() as ctx:
        tc = ctx.enter_context(tile.TileContext(nc))
        const = ctx.enter_context(tc.tile_pool(name="const", bufs=1))
        ppool = ctx.enter_context(tc.tile_pool(name="ppool", bufs=1))
        psq_pool = ctx.enter_context(tc.tile_pool(name="psq", bufs=2))
        invp_pool = ctx.enter_context(tc.tile_pool(name="invp", bufs=1))
        lnp_pool = ctx.enter_context(tc.tile_pool(name="lnp", bufs=2))
        fvec = ctx.enter_context(tc.tile_pool(name="fvec", bufs=NT))
        ftrash = ctx.enter_context(tc.tile_pool(name="ftrash", bufs=2))
        ftb_pool = ctx.enter_context(tc.tile_pool(name="ftbp", bufs=3))
        fnat_pool = ctx.enter_context(tc.tile_pool(name="fnatp", bufs=2))
        stage = ctx.enter_context(tc.tile_pool(name="stage", bufs=4))
        psum = ctx.enter_context(tc.tile_pool(name="psum", bufs=8, space="PSUM"))

        # --- distance_scale vectors -------------------------------------
        ds_one = const.tile([1, 1], f32)
        nc.sync.dma_start(out=ds_one, in_=dsc[:, :])
        ds_bc = const.tile([P, 1], f32)
        nc.gpsimd.partition_broadcast(ds_bc[:, :], ds_one[:, :])
        zero_vec = const.tile([P, 1], f32)
        nc.vector.memset(zero_vec, 0.0)
        ds2 = const.tile([P, 1], f32)
        nc.vector.tensor_mul(ds2[:, :], ds_bc[:, :], ds_bc[:, :])
        neg2ds2 = const.tile([P, 1], f32)  # -2*ds^2
        nc.vector.tensor_scalar_mul(neg2ds2[:, :], ds2[:, :], -2.0)
        bias_vec = const.tile([P, 1], f32)  # +2*ds^2
        nc.vector.tensor_scalar_mul(bias_vec[:, :], ds2[:, :], 2.0)

        ones_bf = const.tile([P, P], bf16)
        nc.vector.memset(ones_bf, 1.0)

        # --- load pT ----------------------------------------------------
        pts = []
        for k in range(KT):
            pt = ppool.tile([P, C], bf16, tag=f"pt{k}", name=f"pt{k}")
            nc.sync.dma_start(out=pt, in_=ptb[k, :, :])
            pts.append(pt)

        # --- f norms ----------------------------------------------------
        scale_vecs = []
        for nt in range(NT):
            ft = fnat_pool.tile([P, D], bf16)
            nc.sync.dma_start(out=ft, in_=fnat[nt, :, :])
            trash = ftrash.tile([P, D], bf16)
            sumsq = fvec.tile([P, 1], f32, tag="sumsq")
            nc.vector.tensor_mul(trash[:, :], ft[:, :], ft[:, :])
            nc.vector.reduce_sum(sumsq[:, :], trash[:, :], axis=mybir.AxisListType.X)
            nc.scalar.activation(
                out=sumsq[:, :], in_=sumsq[:, :], func=AF.Sqrt, bias=zero_vec[:, :]
            )
            nc.vector.reciprocal(out=sumsq[:, :], in_=sumsq[:, :])
            sv = fvec.tile([P, 1], f32, tag="scalevec")
            nc.vector.tensor_mul(sv[:, :], sumsq[:, :], neg2ds2[:, :])
            scale_vecs.append(sv)

        # --- p norms (inv_p broadcast row) + normalize pT ----------------
        invp = invp_pool.tile([P, C], bf16)
        for cg in range(CG):
            c0 = cg * CW
            pinv_psums = []
            for cb in range(CB):
                pinv_psums.append(psum.tile([P, 512], f32, tag="psum", name=f"pinv{cg}_{cb}"))
            for k in range(KT):
                sq = psq_pool.tile([P, CW], bf16)
                nc.vector.tensor_mul(
                    sq[:, :], pts[k][:, c0 : c0 + CW], pts[k][:, c0 : c0 + CW]
                )
                for cb in range(CB):
                    nc.tensor.matmul(
                        pinv_psums[cb],
                        ones_bf[:, :],
                        sq[:, cb * 512 : (cb + 1) * 512],
                        start=(k == 0),
                        stop=(k == KT - 1),
                    )
            for cb in range(CB):
                ln = lnp_pool.tile([P, 512], f32)
                nc.scalar.activation(
                    out=ln[:, :], in_=pinv_psums[cb], func=AF.Ln, bias=zero_vec[:, :]
                )
                nc.scalar.activation(
                    out=invp[:, c0 + cb * 512 : c0 + (cb + 1) * 512],
                    in_=ln[:, :],
                    func=AF.Exp,
                    bias=zero_vec[:, :],
                    scale=-0.5,
                )
            for k in range(KT):
                nc.vector.tensor_mul(
                    pts[k][:, c0 : c0 + CW],
                    pts[k][:, c0 : c0 + CW],
                    invp[:, c0 : c0 + CW],
                )

        # --- main matmul + postprocess ----------------------------------
        for cg in range(CG):
            c0 = cg * CW
            for nt in range(NT):
                ftt = ftb_pool.tile([P, KT, P], bf16)
                nc.sync.dma_start(out=ftt, in_=ftb[nt, :, :, :])
                outs_psum = []
                for cb in range(CB):
                    outs_psum.append(psum.tile([P, 512], f32, tag="psum", name=f"ops{cg}_{nt}_{cb}"))
                for k in range(KT):
                    for cb in range(CB):
                        nc.tensor.matmul(
                            outs_psum[cb],
                            ftt[:, k, :],
                            pts[k][:, c0 + cb * 512 : c0 + (cb + 1) * 512],
                            start=(k == 0),
                            stop=(k == KT - 1),
                        )
                for cb in range(CB):
                    st = stage.tile([P, 512], f32)
                    nc.scalar.activation(
                        out=st[:, :],
                        in_=outs_psum[cb],
                        func=AF.Sqrt,
                        bias=bias_vec[:, :],
                        scale=scale_vecs[nt][:, :],
                    )
                    nc.vector.tensor_scalar_mul(st[:, :], st[:, :], -1.0)
                    nc.sync.dma_start(
                        out=out[
                            nt * P : (nt + 1) * P, c0 + cb * 512 : c0 + (cb + 1) * 512
                        ],
                        in_=st[:, :],
                    )

    nc.finalize()
    return nc


def _get_nc():
    if "nc" not in _ctx:
        _ctx["nc"] = _build_nc()
    return _ctx["nc"]


def kernel(features, prototypes, distance_scale):
    from concourse.bass_utils import run_bass_kernel_spmd

    bf = ml_dtypes.bfloat16
    features = np.asarray(features, dtype=np.float32)
    prototypes = np.asarray(prototypes, dtype=np.float32)
    distance_scale = np.asarray(distance_scale, dtype=np.float32)

    nc = _get_nc()

    # prototypes^T, bf16, tiled over the contraction dim
    ptb_np = np.ascontiguousarray(prototypes.astype(bf).T).reshape(KT, P, C)
    dsc_np = distance_scale.reshape(1, 1)

    in_maps = []
    for core in range(NCORES):
        sh = features[core * NSH : (core + 1) * NSH].astype(bf)
        # [nt, j, k, p] -> [nt, p, k, j]  (lhsT tiles: d on partitions)
        ftb_np = np.ascontiguousarray(sh.reshape(NT, P, KT, P).transpose(0, 3, 2, 1))
        fnat_np = np.ascontiguousarray(sh.reshape(NT, P, D))
        in_maps.append(
            {"ftb": ftb_np, "fnat": fnat_np, "ptb": ptb_np, "dsc": dsc_np}
        )

    res = run_bass_kernel_spmd(nc, in_maps, core_ids=list(range(NCORES)))
    return np.concatenate(
        [res.results[i]["out"] for i in range(NCORES)], axis=0
    ).astype(np.float32)



# revision 5
# speedup vs baseline: 1.7148x; 1.7148x over previous
"""IsoMaxPlus first-part logits kernel for 8 Trainium2 NeuronCores.

reference:
    f = l2norm(features)   [N=16384, D=1024]
    p = l2norm(prototypes) [C=8192, D=1024]
    logits = -|ds| * sqrt(max(2 - 2 * f @ p.T, 1e-12))

Strategy (data-parallel over N, prototypes replicated, fp8 DoubleRow):
  - Host: shard features over 8 cores (2048 rows each); cast both operands
    to fp8e4 (e4m3, TRN flavor) with power-of-two pre-scales that put the
    values mid-range (features x16, prototypes x1600).  Layouts are chosen
    so the TensorEngine sees DoubleRow-ready tiles: contraction dim d on
    partitions, k-tile pairs adjacent in the free dim.
  - Device per core:
      * f row norms: ACT Square with accum_out -> sum(F8^2) per row, ACT
        Sqrt + DVE reciprocal, folded into the post-matmul ACT scale.
      * p col norms: DVE (P8*(1/64))*P8 -> fp8 squares, DoubleRow
        ones-matmul partition-reduce (broadcast over partitions for free),
        ACT Sqrt(x/16) + DVE reciprocal -> inv_p, DVE multiply back into
        the p8 tile in place (renormalized fp8, x32 scale).
      * main matmul: DoubleRow fp8 (K=256 per MM), 8 c-groups of 1024,
        2 PSUM banks per (group, n-tile), accumulated over 4 supertiles.
      * post: ACT Sqrt(svec[m]*dot + 2ds^2) -> bf16, DVE negate (2x mode),
        DMA out bf16; host upcasts to f32.
  - Group-level software pipeline: squares+ones-matmuls for group g+1 are
    emitted before group g's main matmuls so DVE/PE never head-block.
  - max(.., 1e-12) is dropped: 2-2*dot >= 1.5 for this distribution.

Measured end-to-end relative error vs the f32 reference is ~2e-3
(fp8 quantization noise averaged over D=1024), well under the 2e-2 gate.
"""

import sys

import numpy as np
import ml_dtypes

if "/opt/trn_rl_repo" not in sys.path:
    sys.path.append("/opt/trn_rl_repo")

N, D, C = 16384, 1024, 8192
NCORES = 8
NSH = N // NCORES  # rows per core = 2048
P = 128
NT = NSH // P  # 16 n-tiles per core
KT = D // P  # 8 k-tiles of 128
S = KT // 2  # 4 DoubleRow supertiles (K=256 each)
G = 8  # c groups
CW = C // G  # 1024 columns per group
CB = CW // 512  # 2 psum chunks of 512 per group

SF = 16.0  # feature fp8 pre-scale
SP = 1600.0  # prototype fp8 pre-scale

_ctx = {}


def _build_nc():
    import concourse.mybir as mybir
    import concourse.tile as tile
    from concourse import bacc
    from contextlib import ExitStack

    f32 = mybir.dt.float32
    bf16 = mybir.dt.bfloat16
    f8 = mybir.dt.float8e4
    AF = mybir.ActivationFunctionType
    DR = mybir.MatmulPerfMode.DoubleRow
    MUL = mybir.AluOpType.mult

    nc = bacc.Bacc(None, target_bir_lowering=False)

    ftb = nc.dram_tensor("ftb", [NT, P, KT, P], f8, kind="ExternalInput")
    fnat = nc.dram_tensor("fnat", [NT, P, D], f8, kind="ExternalInput")
    ptb = nc.dram_tensor("ptb", [P, G, KT, CW], f8, kind="ExternalInput")
    dsc = nc.dram_tensor("dsc", [1, 1], f32, kind="ExternalInput")
    out = nc.dram_tensor("out", [NSH, C], bf16, kind="ExternalOutput")

    with ExitStack() as ctx:
        tc = ctx.enter_context(tile.TileContext(nc))
        const = ctx.enter_context(tc.tile_pool(name="const", bufs=1))
        ppool = ctx.enter_context(tc.tile_pool(name="ppool", bufs=1))
        sq8_pool = ctx.enter_context(tc.tile_pool(name="sq8", bufs=2))
        invp_pool = ctx.enter_context(tc.tile_pool(name="invp", bufs=3))
        f8_pool = ctx.enter_context(tc.tile_pool(name="f8p", bufs=1))
        fnat_pool = ctx.enter_context(tc.tile_pool(name="fnatp", bufs=3))
        ftrash = ctx.enter_context(tc.tile_pool(name="ftrash", bufs=2))
        fsum_pool = ctx.enter_context(tc.tile_pool(name="fsum", bufs=2))
        svec_pool = ctx.enter_context(tc.tile_pool(name="svec", bufs=1))
        stage = ctx.enter_context(tc.tile_pool(name="stage", bufs=4))
        psum = ctx.enter_context(tc.tile_pool(name="psum", bufs=8, space="PSUM"))

        # --- distance_scale vectors -------------------------------------
        ds_one = const.tile([1, 1], f32)
        nc.sync.dma_start(out=ds_one, in_=dsc[:, :])
        ds_bc = const.tile([P, 1], f32)
        nc.gpsimd.partition_broadcast(ds_bc[:, :], ds_one[:, :])
        ds2 = const.tile([P, 1], f32)
        nc.vector.tensor_mul(ds2[:, :], ds_bc[:, :], ds_bc[:, :])
        bias_vec = const.tile([P, 1], f32)  # +2*ds^2
        nc.vector.tensor_scalar_mul(bias_vec[:, :], ds2[:, :], 2.0)
        negk = const.tile([P, 1], f32)  # -ds^2/16
        nc.vector.tensor_scalar_mul(negk[:, :], ds2[:, :], -1.0 / 16.0)

        ones8 = const.tile([P, 2, P], f8)
        nc.vector.memset(ones8, 1.0)

        # --- input DMAs -------------------------------------------------
        p8t = ppool.tile([P, G, KT, CW], f8, tag="p8t", name="p8t")
        for g in range(G):
            nc.sync.dma_start(out=p8t[:, g, :, :], in_=ptb[:, g, :, :])
        f8ts = []
        for nt in range(NT):
            f8ts.append(f8_pool.tile([P, KT, P], f8, tag=f"f8_{nt}", name=f"f8_{nt}"))
            nc.sync.dma_start(out=f8ts[nt], in_=ftb[nt, :, :, :])

        # --- f norms -> per-row ACT scale vectors -----------------------
        svecs = []
        for nt in range(NT):
            ft = fnat_pool.tile([P, D], f8)
            nc.sync.dma_start(out=ft, in_=fnat[nt, :, :])
            trash = ftrash.tile([P, D], bf16)
            fsum = fsum_pool.tile([P, 1], f32, tag="fsum")
            nc.scalar.activation(
                out=trash[:, :], in_=ft[:, :], func=AF.Square, accum_out=fsum[:, :]
            )
            # fsum = 256*||f||^2 ; fs = 16*||f||
            nc.scalar.activation(out=fsum[:, :], in_=fsum[:, :], func=AF.Sqrt)
            nc.vector.reciprocal(out=fsum[:, :], in_=fsum[:, :])
            sv = svec_pool.tile([P, 1], f32, tag=f"svec{nt}", name=f"svec{nt}")
            nc.vector.tensor_mul(sv[:, :], fsum[:, :], negk[:, :])
            svecs.append(sv)

        # --- p-norm prep stages (group-level software pipeline) ---------
        pinv_psums = {}

        def prep_sq_ones(g):
            """DVE squares + PE DoubleRow ones-matmul partition reduce."""
            sq8 = sq8_pool.tile([P, KT, CW], f8, tag="sq8")
            nc.vector.scalar_tensor_tensor(
                out=sq8[:, :, :],
                in0=p8t[:, g, :, :],
                scalar=1.0 / 64.0,
                in1=p8t[:, g, :, :],
                op0=MUL,
                op1=MUL,
            )
            banks = []
            for cb in range(CB):
                banks.append(psum.tile([P, 512], f32, tag="psum", name=f"pinv{g}_{cb}"))
            for s in range(S):
                for cb in range(CB):
                    nc.tensor.matmul(
                        banks[cb],
                        ones8[:, :, :],
                        sq8[:, 2 * s : 2 * s + 2, cb * 512 : (cb + 1) * 512],
                        start=(s == 0),
                        stop=(s == S - 1),
                        perf_mode=DR,
                    )
            pinv_psums[g] = banks

        def prep_invp_pnt(g):
            """ACT sqrt + DVE reciprocal -> inv_p; renormalize p8 in place."""
            banks = pinv_psums.pop(g)
            invp = invp_pool.tile([P, CW], bf16, tag="invp")
            for cb in range(CB):
                nc.scalar.activation(
                    out=invp[:, cb * 512 : (cb + 1) * 512],
                    in_=banks[cb],
                    func=AF.Sqrt,
                    scale=1.0 / 16.0,
                )
            with nc.allow_low_precision(reason="bf16 inv_p scale, 2^-9 noise ok"):
                nc.vector.reciprocal(out=invp[:, :], in_=invp[:, :])
            for kt in range(KT):
                nc.vector.tensor_mul(
                    p8t[:, g, kt, :], p8t[:, g, kt, :], invp[:, :]
                )

        def main_group(g):
            for nt in range(NT):
                banks = []
                for cb in range(CB):
                    banks.append(
                        psum.tile([P, 512], f32, tag="psum", name=f"ops{g}_{nt}_{cb}")
                    )
                for s in range(S):
                    for cb in range(CB):
                        nc.tensor.matmul(
                            banks[cb],
                            f8ts[nt][:, 2 * s : 2 * s + 2, :],
                            p8t[:, g, 2 * s : 2 * s + 2, cb * 512 : (cb + 1) * 512],
                            start=(s == 0),
                            stop=(s == S - 1),
                            perf_mode=DR,
                        )
                st = stage.tile([P, CW], bf16)
                for cb in range(CB):
                    nc.scalar.activation(
                        out=st[:, cb * 512 : (cb + 1) * 512],
                        in_=banks[cb],
                        func=AF.Sqrt,
                        bias=bias_vec[:, :],
                        scale=svecs[nt][:, :],
                    )
                nc.vector.tensor_scalar_mul(st[:, :], st[:, :], -1.0)
                nc.sync.dma_start(
                    out=out[nt * P : (nt + 1) * P, g * CW : (g + 1) * CW],
                    in_=st[:, :],
                )

        prep_sq_ones(0)
        for g in range(G):
            prep_invp_pnt(g)
            if g + 1 < G:
                prep_sq_ones(g + 1)
            main_group(g)

    nc.finalize()
    return nc


def _get_nc():
    if "nc" not in _ctx:
        _ctx["nc"] = _build_nc()
    return _ctx["nc"]


def make_in_maps(features, prototypes, distance_scale):
    """Host-side shard + fp8 cast + layout. No arithmetic beyond scaling."""
    f8 = ml_dtypes.float8_e4m3
    features = np.asarray(features, dtype=np.float32)
    prototypes = np.asarray(prototypes, dtype=np.float32)
    distance_scale = np.asarray(distance_scale, dtype=np.float32)

    # prototypes^T, fp8, group-major tiling: [P, G, KT, CW]
    ptb_np = np.ascontiguousarray(
        (prototypes.T * SP)
        .astype(f8)
        .reshape(KT, P, G, CW)
        .transpose(1, 2, 0, 3)
    )
    dsc_np = distance_scale.reshape(1, 1)

    in_maps = []
    for core in range(NCORES):
        sh = (features[core * NSH : (core + 1) * NSH] * SF).astype(f8)
        # [nt, j, k, p] -> [nt, p, k, j]  (lhsT tiles: d on partitions)
        ftb_np = np.ascontiguousarray(sh.reshape(NT, P, KT, P).transpose(0, 3, 2, 1))
        fnat_np = np.ascontiguousarray(sh.reshape(NT, P, D))
        in_maps.append({"ftb": ftb_np, "fnat": fnat_np, "ptb": ptb_np, "dsc": dsc_np})
    return in_maps


def kernel(features, prototypes, distance_scale):
    from concourse.bass_utils import run_bass_kernel_spmd

    nc = _get_nc()
    in_maps = make_in_maps(features, prototypes, distance_scale)
    res = run_bass_kernel_spmd(nc, in_maps, core_ids=list(range(NCORES)))
    return np.concatenate(
        [np.asarray(res.results[i]["out"]) for i in range(NCORES)], axis=0
    ).astype(np.float32)


# revision 7
# speedup vs baseline: 2.2525x; 1.3136x over previous
"""IsoMaxPlus first-part logits kernel for 8 Trainium2 NeuronCores.

reference:
    f = l2norm(features)   [N=16384, D=1024]
    p = l2norm(prototypes) [C=8192, D=1024]
    logits = -|ds| * sqrt(max(2 - 2 * f @ p.T, 1e-12))

Strategy (data-parallel over N, prototypes replicated, fp8 DoubleRow):
  - Host: shard features over 8 cores (2048 rows each); cast both operands
    to fp8e4 (e4m3) with pre-scales that put values mid-range (features
    x16, prototypes x1600). Layouts are DoubleRow-ready: contraction dim d
    on partitions, k-tile pairs adjacent in the free dim.
  - Device per core:
      * f row norms: ACT Square + accum_out, batched Sqrt + fast DVE
        reciprocal, folded into the post-matmul ACT scale.
      * p col norms: DVE (P8*(1/64))*P8 fp8 squares, DoubleRow ones-matmul
        partition-reduce (result broadcast over partitions for free), ACT
        Sqrt(x/16) + DVE reciprocal_approx_fast -> inv_p (f32), DVE
        multiply back into the p8 tile in place (renormalized fp8, x32).
      * main matmul: DoubleRow fp8 (K=256 per MM), 8 c-groups of 1024,
        2 PSUM banks per (group, n-tile), accumulated over 4 supertiles.
      * post: ACT Sqrt(svec[m]*dot + 2ds^2) -> bf16, DVE negate (4x mode),
        DMA out bf16; host upcasts to f32.
  - Software pipeline: squares for group g+2 and the ones-matmul /
    inv_p / renormalize chain for group g+1 are emitted at staggered
    n-tile offsets inside group g's main loop, so PE never waits and no
    engine FIFO head-blocks.
  - max(.., 1e-12) is dropped: 2-2*dot >= 1.5 for this distribution.

Measured end-to-end relative error vs the f32 reference is ~7e-3
(fp8 quantization noise averaged over D=1024), under the 2e-2 gate.
"""

import sys

import numpy as np
import ml_dtypes

if "/opt/trn_rl_repo" not in sys.path:
    sys.path.append("/opt/trn_rl_repo")

N, D, C = 16384, 1024, 8192
NCORES = 8
NSH = N // NCORES  # rows per core = 2048
P = 128
NT = NSH // P  # 16 n-tiles per core
KT = D // P  # 8 k-tiles of 128
S = KT // 2  # 4 DoubleRow supertiles (K=256 each)
G = 8  # c groups
CW = C // G  # 1024 columns per group
CB = CW // 512  # 2 psum chunks of 512 per group

SF = 16.0  # feature fp8 pre-scale
SP = 1600.0  # prototype fp8 pre-scale

_ctx = {}


def _build_nc():
    import concourse.mybir as mybir
    import concourse.tile as tile
    from concourse import bacc
    from contextlib import ExitStack

    f32 = mybir.dt.float32
    bf16 = mybir.dt.bfloat16
    f8 = mybir.dt.float8e4
    AF = mybir.ActivationFunctionType
    DR = mybir.MatmulPerfMode.DoubleRow
    MUL = mybir.AluOpType.mult

    nc = bacc.Bacc(None, target_bir_lowering=False)

    ftb = nc.dram_tensor("ftb", [NT, P, KT, P], f8, kind="ExternalInput")
    fnat = nc.dram_tensor("fnat", [NT, P, D], f8, kind="ExternalInput")
    ptb = nc.dram_tensor("ptb", [P, G, KT, CW], f8, kind="ExternalInput")
    dsc = nc.dram_tensor("dsc", [1, 1], f32, kind="ExternalInput")
    out = nc.dram_tensor("out", [NSH, C], bf16, kind="ExternalOutput")

    with ExitStack() as ctx:
        tc = ctx.enter_context(tile.TileContext(nc))
        const = ctx.enter_context(tc.tile_pool(name="const", bufs=1))
        ppool = ctx.enter_context(tc.tile_pool(name="ppool", bufs=1))
        sq8_pool = ctx.enter_context(tc.tile_pool(name="sq8", bufs=2))
        invp_pool = ctx.enter_context(tc.tile_pool(name="invp", bufs=2))
        f8_pool = ctx.enter_context(tc.tile_pool(name="f8p", bufs=1))
        fnat_pool = ctx.enter_context(tc.tile_pool(name="fnatp", bufs=1))
        ftrash = ctx.enter_context(tc.tile_pool(name="ftrash", bufs=2))
        stage = ctx.enter_context(tc.tile_pool(name="stage", bufs=10))
        psum = ctx.enter_context(tc.tile_pool(name="psum", bufs=8, space="PSUM"))

        # --- distance_scale vectors -------------------------------------
        ds_one = const.tile([1, 1], f32)
        nc.sync.dma_start(out=ds_one, in_=dsc[:, :])
        ds_bc = const.tile([P, 1], f32)
        nc.gpsimd.partition_broadcast(ds_bc[:, :], ds_one[:, :])
        ds2 = const.tile([P, 1], f32)
        nc.vector.tensor_mul(ds2[:, :], ds_bc[:, :], ds_bc[:, :])
        bias_vec = const.tile([P, 1], f32)  # +2*ds^2
        nc.vector.tensor_scalar_mul(bias_vec[:, :], ds2[:, :], 2.0)
        negk = const.tile([P, 1], f32)  # -ds^2/16
        nc.vector.tensor_scalar_mul(negk[:, :], ds2[:, :], -1.0 / 16.0)

        ones8 = const.tile([P, 2, P], f8)
        nc.vector.memset(ones8, 1.0)

        # --- input DMAs (order matters: g0 first, then f, then rest) ----
        p8t = ppool.tile([P, G, KT, CW], f8, tag="p8t", name="p8t")
        nc.sync.dma_start(out=p8t[:, 0, :, :], in_=ptb[:, 0, :, :])
        fnat_t = fnat_pool.tile([P, NT, D], f8, tag="fnat", name="fnat")
        for nt in range(NT):
            nc.sync.dma_start(out=fnat_t[:, nt, :], in_=fnat[nt, :, :])
        f8ts = []
        for nt in range(NT):
            f8ts.append(f8_pool.tile([P, KT, P], f8, tag=f"f8_{nt}", name=f"f8_{nt}"))
            nc.sync.dma_start(out=f8ts[nt], in_=ftb[nt, :, :, :])
        for g in range(1, G):
            nc.sync.dma_start(out=p8t[:, g, :, :], in_=ptb[:, g, :, :])

        # --- f norm state -----------------------------------------------
        fsum = const.tile([P, NT], f32)
        frec = const.tile([P, NT], f32)
        svec = const.tile([P, NT], f32)

        def f_squares(lo, hi):
            for nt in range(lo, hi):
                trash = ftrash.tile([P, D], bf16)
                nc.scalar.activation(
                    out=trash[:, :],
                    in_=fnat_t[:, nt, :],
                    func=AF.Square,
                    accum_out=fsum[:, nt : nt + 1],
                )

        def f_finish(lo, hi):
            # fsum = 256*||f||^2 -> svec = -ds^2/(256*||f||)
            nc.scalar.activation(
                out=fsum[:, lo:hi], in_=fsum[:, lo:hi], func=AF.Sqrt
            )
            nc.vector.reciprocal_approx_fast(out=frec[:, lo:hi], in_=fsum[:, lo:hi])
            nc.vector.tensor_scalar_mul(svec[:, lo:hi], frec[:, lo:hi], negk[:, :])

        # --- p-norm pipeline stages -------------------------------------
        sq8_tiles = {}
        pinv_psums = {}
        invp_tiles = {}

        def p_squares(g):
            sq8 = sq8_pool.tile([P, KT, CW], f8, tag="sq8")
            nc.vector.scalar_tensor_tensor(
                out=sq8[:, :, :],
                in0=p8t[:, g, :, :],
                scalar=1.0 / 64.0,
                in1=p8t[:, g, :, :],
                op0=MUL,
                op1=MUL,
            )
            sq8_tiles[g] = sq8

        def p_ones_mm(g):
            sq8 = sq8_tiles.pop(g)
            banks = []
            for cb in range(CB):
                banks.append(psum.tile([P, 512], f32, tag="psum", name=f"pinv{g}_{cb}"))
            for s in range(S):
                for cb in range(CB):
                    nc.tensor.matmul(
                        banks[cb],
                        ones8[:, :, :],
                        sq8[:, 2 * s : 2 * s + 2, cb * 512 : (cb + 1) * 512],
                        start=(s == 0),
                        stop=(s == S - 1),
                        perf_mode=DR,
                    )
            pinv_psums[g] = banks

        def p_sqrt(g):
            banks = pinv_psums.pop(g)
            invp = invp_pool.tile([P, CW], f32, tag="invp")
            for cb in range(CB):
                nc.scalar.activation(
                    out=invp[:, cb * 512 : (cb + 1) * 512],
                    in_=banks[cb],
                    func=AF.Sqrt,
                    scale=1.0 / 16.0,
                )
            invp_tiles[g] = invp

        def p_recip(g):
            invp = invp_tiles[g]
            nc.vector.reciprocal_approx_fast(out=invp[:, :], in_=invp[:, :])

        def p_renorm(g):
            invp = invp_tiles.pop(g)
            for kt in range(KT):
                nc.vector.tensor_mul(p8t[:, g, kt, :], p8t[:, g, kt, :], invp[:, :])

        def main_group(g):
            for nt in range(NT):
                if nt == 2 and g + 2 < G:
                    p_squares(g + 2)
                if nt == 6 and g + 1 < G:
                    p_ones_mm(g + 1)
                banks = []
                for cb in range(CB):
                    banks.append(
                        psum.tile([P, 512], f32, tag="psum", name=f"ops{g}_{nt}_{cb}")
                    )
                for s in range(S):
                    for cb in range(CB):
                        nc.tensor.matmul(
                            banks[cb],
                            f8ts[nt][:, 2 * s : 2 * s + 2, :],
                            p8t[:, g, 2 * s : 2 * s + 2, cb * 512 : (cb + 1) * 512],
                            start=(s == 0),
                            stop=(s == S - 1),
                            perf_mode=DR,
                        )
                st = stage.tile([P, CW], bf16)
                for cb in range(CB):
                    nc.scalar.activation(
                        out=st[:, cb * 512 : (cb + 1) * 512],
                        in_=banks[cb],
                        func=AF.Sqrt,
                        bias=bias_vec[:, :],
                        scale=svec[:, nt : nt + 1],
                    )
                if nt == 6 and g + 1 < G:
                    p_sqrt(g + 1)
                nc.vector.tensor_scalar_mul(st[:, :], st[:, :], -1.0)
                if nt == 7 and g + 1 < G:
                    p_recip(g + 1)
                if nt == 8 and g + 1 < G:
                    p_renorm(g + 1)
                nc.sync.dma_start(
                    out=out[nt * P : (nt + 1) * P, g * CW : (g + 1) * CW],
                    in_=st[:, :],
                )

        # --- prologue: group 0 prep + f norms, interleaved so the ACT
        # f-squares fill the DMA window without delaying the p-chain, and
        # the DVE renormalize of group 0 starts as early as possible.
        p_squares(0)  # DVE, gated only on the group-0 DMA
        p_ones_mm(0)  # PE
        f_squares(0, 8)  # ACT, runs while the ones-matmul completes
        p_sqrt(0)  # ACT
        f_finish(0, 8)  # ACT tiny + DVE tiny
        p_recip(0)  # DVE
        p_renorm(0)  # DVE -> unblocks main(0)
        f_squares(8, NT)  # ACT
        p_squares(1)  # DVE
        f_finish(8, NT)

        for g in range(G):
            main_group(g)

    nc.finalize()
    return nc


def _get_nc():
    if "nc" not in _ctx:
        _ctx["nc"] = _build_nc()
    return _ctx["nc"]


def make_in_maps(features, prototypes, distance_scale):
    """Host-side shard + fp8 cast + layout. No arithmetic beyond scaling."""
    f8 = ml_dtypes.float8_e4m3
    features = np.asarray(features, dtype=np.float32)
    prototypes = np.asarray(prototypes, dtype=np.float32)
    distance_scale = np.asarray(distance_scale, dtype=np.float32)

    # prototypes^T, fp8, group-major tiling: [P, G, KT, CW]
    ptb_np = np.ascontiguousarray(
        (prototypes.T * SP)
        .astype(f8)
        .reshape(KT, P, G, CW)
        .transpose(1, 2, 0, 3)
    )
    dsc_np = distance_scale.reshape(1, 1)

    in_maps = []
    for core in range(NCORES):
        sh = (features[core * NSH : (core + 1) * NSH] * SF).astype(f8)
        # [nt, j, k, p] -> [nt, p, k, j]  (lhsT tiles: d on partitions)
        ftb_np = np.ascontiguousarray(sh.reshape(NT, P, KT, P).transpose(0, 3, 2, 1))
        fnat_np = np.ascontiguousarray(sh.reshape(NT, P, D))
        in_maps.append({"ftb": ftb_np, "fnat": fnat_np, "ptb": ptb_np, "dsc": dsc_np})
    return in_maps


def kernel(features, prototypes, distance_scale):
    from concourse.bass_utils import run_bass_kernel_spmd

    nc = _get_nc()
    in_maps = make_in_maps(features, prototypes, distance_scale)
    res = run_bass_kernel_spmd(nc, in_maps, core_ids=list(range(NCORES)))
    return np.concatenate(
        [np.asarray(res.results[i]["out"]) for i in range(NCORES)], axis=0
    ).astype(np.float32)
